# revision 37
# baseline (speedup 1.0000x reference)
"""Trainium2 Bass kernel for causal self-attention with RoPE (tensor-parallel over 8 cores).

Contract: kernel(**inputs) takes full unsharded inputs (x, W_attn, b_attn,
W_proj, b_proj), shards across 8 NeuronCores (2 heads each), runs one SPMD
Bass/Tile kernel, and host-reduces the partial c_proj outputs.

Design notes (HW-measured best of 10 structural variants, ~433us vs 452us
baseline on core 0):
- RoPE entirely on DVE via partition-shifted reads of the chain psum with
  a sign-folded sin table (replaces 64 rotation matmuls + 32 scalar
  copies of the baseline).
- Causal column restriction: diagonal key-blocks only compute score/exp/
  attV/Z columns >= c0; a single shared [128,128] triangle mask handles
  the block-diagonal boundary (~15% less attention work than full-block).
- Softmax denominator Z accumulated on DVE (f32), finalized with a gpsimd
  partition_all_reduce (replaces 160 [1,512] PE matmuls, ~59us of PE).
- y-psum evacuated promptly via scalar copy (releases the bank without
  waiting on the DVE queue); 1/Z applied in a deferred DVE multiply.
- Diagonal-block attV/Z split so only the [128,128] masked window waits
  on the triangle-mask DVE op; the unmasked span proceeds immediately.
- Heads interleaved per q-tile with double-buffered y-PSUM.
- PSUM: qkv/v chains 2 banks, score pairs 2x[128,1024] 4 banks, y 2.
"""

import os
import sys

import numpy as np

for _p in ("/opt/trn_rl_repo",):
    if os.path.isdir(_p) and _p not in sys.path:
        sys.path.insert(0, _p)

import ml_dtypes
from contextlib import ExitStack

import concourse.bass as bass
import concourse.tile as tile
from concourse import bacc, bass_isa, mybir
from concourse.bass_utils import run_bass_kernel_spmd

# ---- problem constants (hardcoded per contract) ----
B, T, C = 2, 2048, 2048
H, D = 16, 128
N_CORES = 8
HPC = H // N_CORES  # heads per core = 2
ROPE_BASE = 10000.0
SCALE = float(1.0 / np.sqrt(D))
TQ = 512            # query tile (free dim of scores matmul)
NTQ = T // TQ       # 4
TK = 128            # key tile (partition dim of scoresT)
NTK = T // TK       # 16
NCT = C // 128      # 16 contraction tiles for projections
BT = B * T
HD = D // 2         # rope half

F32 = mybir.dt.float32
BF16 = mybir.dt.bfloat16

ADD = mybir.AluOpType.add
MULT = mybir.AluOpType.mult
EXP = mybir.ActivationFunctionType.Exp

PAIR_LOOKAHEAD = 2  # score-pairs ahead of attV in the attention pipeline


def _build_program(with_bias_qk: bool, with_bias_v: bool):
    nc = bacc.Bacc(
        "TRN2", target_bir_lowering=False, debug=False, num_devices=N_CORES
    )

    xT = nc.dram_tensor("xT", [C, BT], BF16, kind="ExternalInput").ap()
    wqk = nc.dram_tensor("wqk", [128, NCT, 4 * D], BF16, kind="ExternalInput").ap()
    wv = nc.dram_tensor("wv", [128, NCT, HPC * D], BF16, kind="ExternalInput").ap()
    wpr = nc.dram_tensor("wpr", [128, HPC, C], BF16, kind="ExternalInput").ap()
    bqk = nc.dram_tensor("bqk", [128, 4], F32, kind="ExternalInput").ap()
    bqkr = nc.dram_tensor("bqkr", [128, 4], F32, kind="ExternalInput").ap()
    bv = nc.dram_tensor("bv", [HPC * D], F32, kind="ExternalInput").ap()
    cosT = nc.dram_tensor("cosT", [D, T], F32, kind="ExternalInput").ap()
    sinNT = nc.dram_tensor("sinNT", [D, T], F32, kind="ExternalInput").ap()
    tri = nc.dram_tensor("tri", [128, 128], BF16, kind="ExternalInput").ap()
    out = nc.dram_tensor("out", [BT, C], BF16, kind="ExternalOutput").ap()

    with tile.TileContext(nc) as tc, ExitStack() as ctx:
        consts = ctx.enter_context(tc.tile_pool(name="consts", bufs=1))
        xt_pool = ctx.enter_context(tc.tile_pool(name="xt", bufs=1))
        qk_pool = ctx.enter_context(tc.tile_pool(name="qk", bufs=1))
        v_pool = ctx.enter_context(tc.tile_pool(name="v", bufs=1))
        e_pool = ctx.enter_context(tc.tile_pool(name="e", bufs=6))
        r_pool = ctx.enter_context(tc.tile_pool(name="rp", bufs=2))
        z_pool = ctx.enter_context(tc.tile_pool(name="zs", bufs=3))
        yn_pool = ctx.enter_context(tc.tile_pool(name="yn", bufs=1))
        ob_pool = ctx.enter_context(tc.tile_pool(name="ob", bufs=3))
        ps_mm = ctx.enter_context(tc.tile_pool(name="ps_mm", bufs=2, space="PSUM"))
        ps_s = ctx.enter_context(tc.tile_pool(name="ps_s", bufs=2, space="PSUM"))
        ps_y = ctx.enter_context(tc.tile_pool(name="ps_y", bufs=2, space="PSUM"))

        # ---- initial loads, interleaved in cold-start consumption order:
        # the cold loop eats (wqk[ct], strip[ct]) every ~0.9us, so small
        # leading wqk chunks + strips staggered across the 3 queues ----
        qs = [nc.sync, nc.gpsimd, nc.scalar]
        wqk_sb = consts.tile([128, NCT, 4 * D], BF16)

        def load_strip(xt_sb, b, ct, q):
            q.dma_start(
                xt_sb[:, ct, :],
                xT[ct * 128 : (ct + 1) * 128, b * T : (b + 1) * T],
            )

        xt_b0 = xt_pool.tile([128, NCT, T], BF16, tag="xt", name="xt_b0")
        nc.sync.dma_start(wqk_sb[:, 0:2, :], wqk[:, 0:2, :])
        nc.gpsimd.dma_start(wqk_sb[:, 2:5, :], wqk[:, 2:5, :])
        load_strip(xt_b0, 0, 2, nc.scalar)
        load_strip(xt_b0, 0, 0, nc.sync)
        load_strip(xt_b0, 0, 1, nc.gpsimd)
        load_strip(xt_b0, 0, 5, nc.scalar)
        nc.gpsimd.dma_start(wqk_sb[:, 5:9, :], wqk[:, 5:9, :])
        nc.scalar.dma_start(wqk_sb[:, 9:16, :], wqk[:, 9:16, :])
        for k, ct in enumerate((3, 6, 9, 12, 15)):
            load_strip(xt_b0, 0, ct, nc.sync)
        for ct in (4, 7, 10, 13):
            load_strip(xt_b0, 0, ct, nc.gpsimd)
        for ct in (8, 11, 14):
            load_strip(xt_b0, 0, ct, nc.scalar)

        cos_sb = consts.tile([128, T], F32)
        nc.sync.dma_start(cos_sb[:], cosT[:])
        sin_sb = consts.tile([128, T], F32)
        nc.gpsimd.dma_start(sin_sb[:], sinNT[:])
        tri_sb = consts.tile([128, 128], BF16)
        nc.scalar.dma_start(tri_sb[:], tri[:])
        wv_sb = consts.tile([128, NCT, HPC * D], BF16)
        nc.scalar.dma_start(wv_sb[:], wv[:])
        wpr_sb = consts.tile([128, HPC, C], BF16)
        nc.sync.dma_start(wpr_sb[:], wpr[:])
        if with_bias_qk:
            bqk_sb = consts.tile([128, 4], F32)
            nc.gpsimd.dma_start(bqk_sb[:], bqk[:])
        if with_bias_v:
            bv_sb = consts.tile([128, HPC * D], F32)
            nc.gpsimd.dma_start(bv_sb[:], bv.to_broadcast((128, HPC * D)))

        def emit_rope(f, t, w, ps, qk_tiles):
            """Matmul-free rope over w cols starting at q-tile t:
            qk[f][:, tsl] = (q+b)*cos + rot_half(q+b)*sinN.
            All four passes run on DVE; the shifted-base reads are legal
            because in0 is PSUM."""
            tsl = slice(t * TQ, t * TQ + w)
            b_all = bqk_sb[:, f : f + 1] if with_bias_qk else 0.0
            b_lo = bqk_sb[0:HD, f : f + 1] if with_bias_qk else 0.0
            b_hi = bqk_sb[HD:D, f : f + 1] if with_bias_qk else 0.0
            t1 = r_pool.tile([128, 2 * TQ], F32, tag="r1")
            nc.vector.scalar_tensor_tensor(
                t1[:, 0:w], ps[:, 0:w], b_all, cos_sb[:, tsl], op0=ADD, op1=MULT
            )
            t2 = r_pool.tile([128, 2 * TQ], F32, tag="r2")
            nc.vector.scalar_tensor_tensor(
                t2[0:HD, 0:w], ps[HD:D, 0:w], b_hi, sin_sb[0:HD, tsl],
                op0=ADD, op1=MULT,
            )
            nc.vector.scalar_tensor_tensor(
                t2[HD:D, 0:w], ps[0:HD, 0:w], b_lo, sin_sb[HD:D, tsl],
                op0=ADD, op1=MULT,
            )
            nc.gpsimd.tensor_add(qk_tiles[f][:, tsl], t1[:, 0:w], t2[:, 0:w])

        def qkv_phase(b, xt_sb):
            """QKV projections + RoPE for batch b. Returns (qk_tiles, v_sb)."""
            # q/k feature tiles: 0=q_h0, 1=q_h1, 2=k_h0, 3=k_h1
            qk_tiles = [
                qk_pool.tile([128, T], BF16, tag=f"qk{f}", name=f"qkt{f}")
                for f in range(4)
            ]
            if b == 0:
                # cold start: t=0 for all four f-tiles ct-major so the PE
                # consumes xT strips as the initial DMAs land.
                cold_a = ps_s.tile([128, 2 * TQ], F32, tag="s", name="cold_a")
                cold_b = ps_s.tile([128, 2 * TQ], F32, tag="s", name="cold_b")
                t0_ps = [
                    cold_a[:, 0:TQ], cold_a[:, TQ : 2 * TQ],
                    cold_b[:, 0:TQ], cold_b[:, TQ : 2 * TQ],
                ]
                for ct in range(NCT):
                    for f in range(4):
                        nc.tensor.matmul(
                            t0_ps[f],
                            wqk_sb[:, ct, f * 128 : (f + 1) * 128],
                            xt_sb[:, ct, 0:TQ],
                            start=(ct == 0),
                            stop=(ct == NCT - 1),
                        )
                for f in range(4):
                    emit_rope(f, 0, TQ, t0_ps[f], qk_tiles)
            for f in range(4):
                for t in range(NTQ):
                    if b == 0 and t == 0:
                        continue
                    ps = ps_mm.tile([128, TQ], F32, tag="mm")
                    for ct in range(NCT):
                        nc.tensor.matmul(
                            ps[:],
                            wqk_sb[:, ct, f * 128 : (f + 1) * 128],
                            xt_sb[:, ct, t * TQ : (t + 1) * TQ],
                            start=(ct == 0),
                            stop=(ct == NCT - 1),
                        )
                    emit_rope(f, t, TQ, ps, qk_tiles)

            # V in [t, d] layout: lhsT = xT tile (c, t), rhs = Wv (c, d)
            v_sb = v_pool.tile([128, NTK, HPC * D], BF16, tag="v")
            for mt in range(NTK):
                ps = ps_mm.tile([128, HPC * D], F32, tag="mm")
                for ct in range(NCT):
                    nc.tensor.matmul(
                        ps[:],
                        xt_sb[:, ct, mt * 128 : (mt + 1) * 128],
                        wv_sb[:, ct, :],
                        start=(ct == 0),
                        stop=(ct == NCT - 1),
                    )
                if with_bias_v:
                    nc.vector.tensor_add(v_sb[:, mt, :], ps[:], bv_sb[:])
                else:
                    nc.scalar.copy(v_sb[:, mt, :], ps[:])
            return qk_tiles, v_sb

        def attention(b, qk_tiles, v_sb):
            """Flash-style causal attention, heads interleaved per q-tile.

            Returns yn tiles ([d, T] bf16, one per head)."""
            yn_h = [
                yn_pool.tile([128, T], BF16, tag=f"yn{hl}", name=f"yn{hl}")
                for hl in range(HPC)
            ]
            fin_backlog = []

            def emit_finalize(yps, zacc, hl, jsl):
                # evacuate yps promptly via scalar (its queue is right
                # behind this unit's exps) so the y-psum slot recycles
                # without waiting on the clogged DVE queue
                ysb = z_pool.tile([128, TQ], F32, tag="ysb", bufs=2)
                nc.scalar.copy(ysb[:], yps[:])
                zsum = z_pool.tile([128, TQ], F32, tag="zsum", bufs=2)
                nc.gpsimd.partition_all_reduce(
                    zsum[:], zacc[:], channels=128, reduce_op=bass_isa.ReduceOp.add
                )
                return (ysb, zsum, hl, jsl)

            def drain_finalize(ysb, zsum, hl, jsl):
                zrec = z_pool.tile([128, TQ], F32, tag="zrec", bufs=2)
                nc.vector.reciprocal_approx_fast(zrec[:], zsum[:])
                nc.vector.tensor_mul(yn_h[hl][:, jsl], ysb[:], zrec[:])

            for j in range(NTQ):
                jsl = slice(j * TQ, (j + 1) * TQ)
                nblk = 4 * j + 4
                # pairs of key-blocks: (i0, c0_of_i0, c0_of_i1); c0 = first
                # valid scores column (block-local) for causality.
                pairs = [(2 * p, 0, 0) for p in range(2 * j)]
                pairs.append((4 * j, 0, 128))
                pairs.append((4 * j + 2, 256, 384))
                npair = len(pairs)
                for hl in range(HPC):
                    qT = qk_tiles[hl]
                    kT = qk_tiles[2 + hl]
                    yps = ps_y.tile([128, TQ], F32, tag="y")
                    zacc = z_pool.tile([128, TQ], F32, tag="zacc")
                    e_tiles = [None] * npair

                    def emit_pair(p):
                        i0, c00, c01 = pairs[p]
                        sps = ps_s.tile([128, 2 * TQ], F32, tag="s")
                        for u, c0 in ((0, c00), (1, c01)):
                            i = i0 + u
                            nc.tensor.matmul(
                                sps[:, u * TQ + c0 : (u + 1) * TQ],
                                kT[:, i * TK : (i + 1) * TK],
                                qT[:, j * TQ + c0 : (j + 1) * TQ],
                                start=True,
                                stop=True,
                            )
                        e = e_pool.tile([128, 2 * TQ], BF16, tag="e")
                        # one exp over [c00 : 1024]; the gap columns
                        # [TQ : TQ+c01) hold garbage that is never read.
                        nc.scalar.activation(
                            e[:, c00:], sps[:, c00:], EXP, bias=0.0, scale=SCALE
                        )
                        e_tiles[p] = e

                    def emit_consume(p):
                        i0, c00, c01 = pairs[p]
                        e = e_tiles[p]
                        if c00 == 0 and c01 == 0:
                            # full pair: bf16 leaf sum (2x DVE rate), one
                            # f32 fold into zacc instead of two
                            zt = z_pool.tile([128, TQ], BF16, tag="zt",
                                             bufs=2)
                            nc.vector.tensor_add(
                                zt[:], e[:, 0:TQ], e[:, TQ : 2 * TQ]
                            )
                            if i0 == 0:
                                nc.vector.tensor_copy(zacc[:], zt[:])
                            else:
                                nc.vector.tensor_add(zacc[:], zacc[:], zt[:])
                            for u in range(2):
                                i = i0 + u
                                nc.tensor.matmul(
                                    yps[:],
                                    v_sb[:, i, hl * D : (hl + 1) * D],
                                    e[:, u * TQ : (u + 1) * TQ],
                                    start=(i == 0),
                                    stop=(i == nblk - 1),
                                )
                            return
                        for u, c0 in ((0, c00), (1, c01)):
                            i = i0 + u
                            vi = v_sb[:, i, hl * D : (hl + 1) * D]
                            first = i == 0
                            last = i == nblk - 1
                            if i < 4 * j or first:
                                # full block, or the group-opening block
                                # (j==0 r0): single start=True matmul so the
                                # psum init covers one contiguous region.
                                if first and i >= 4 * j:
                                    ew0 = e[:, u * TQ : u * TQ + 128]
                                    nc.vector.tensor_mul(ew0, ew0, tri_sb[:])
                                eh = e[:, u * TQ : (u + 1) * TQ]
                                if first:
                                    nc.vector.tensor_copy(zacc[:], eh)
                                else:
                                    nc.vector.tensor_add(zacc[:], zacc[:], eh)
                                nc.tensor.matmul(
                                    yps[:], vi, eh, start=first, stop=last
                                )
                                continue
                            # diagonal block: the unmasked span [c0+128:TQ]
                            # proceeds without waiting on the mask; only the
                            # [128,128] masked window is gated on DVE.
                            whi = c0 + 128
                            if whi < TQ:
                                ehb = e[:, u * TQ + whi : (u + 1) * TQ]
                                nc.vector.tensor_add(
                                    zacc[:, whi:], zacc[:, whi:], ehb
                                )
                                nc.tensor.matmul(
                                    yps[:, whi:], vi, ehb,
                                    start=False, stop=False,
                                )
                            ew = e[:, u * TQ + c0 : u * TQ + whi]
                            nc.vector.tensor_mul(ew, ew, tri_sb[:])
                            nc.vector.tensor_add(
                                zacc[:, c0:whi], zacc[:, c0:whi], ew
                            )
                            nc.tensor.matmul(
                                yps[:, c0:whi], vi, ew, start=False, stop=last
                            )

                    for p in range(npair):
                        emit_pair(p)
                        if p >= PAIR_LOOKAHEAD:
                            emit_consume(p - PAIR_LOOKAHEAD)
                    for p in range(max(0, npair - PAIR_LOOKAHEAD), npair):
                        emit_consume(p)

                    fin_backlog.append(emit_finalize(yps, zacc, hl, jsl))
                    # drain the previous (j,hl)'s finalize now: its gpsimd
                    # all-reduce has had a full head-slot to complete, so the
                    # DVE queue won't stall on it.
                    if len(fin_backlog) > 1:
                        drain_finalize(*fin_backlog.pop(0))
            while fin_backlog:
                drain_finalize(*fin_backlog.pop(0))
            return yn_h

        def cproj_phase(b, yn_h):
            oq = [nc.sync, nc.gpsimd]
            for mt in range(NTK):
                osb = ob_pool.tile([128, C], BF16, tag="ob")
                for np_ in range(NTQ // 2):
                    ops = ps_s.tile([128, 2 * TQ], F32, tag="s")
                    for u in range(2):
                        n = 2 * np_ + u
                        nsl_ps = slice(u * TQ, (u + 1) * TQ)
                        for hl in range(HPC):
                            nc.tensor.matmul(
                                ops[:, nsl_ps],
                                yn_h[hl][:, mt * 128 : (mt + 1) * 128],
                                wpr_sb[:, hl, n * TQ : (n + 1) * TQ],
                                start=(hl == 0),
                                stop=(hl == HPC - 1),
                            )
                    osl = slice(2 * np_ * TQ, 2 * (np_ + 1) * TQ)
                    if np_ % 2 == 0:
                        nc.vector.tensor_copy(osb[:, osl], ops[:])
                    else:
                        nc.scalar.copy(osb[:, osl], ops[:])
                oq[mt % 2].dma_start(
                    out[b * T + mt * 128 : b * T + (mt + 1) * 128, :], osb[:]
                )

        xt_sb = xt_b0
        for b in range(B):
            qk_tiles, v_sb = qkv_phase(b, xt_sb)
            if b + 1 < B:
                xt_sb = xt_pool.tile([128, NCT, T], BF16, tag="xt", name="xt_b1")
                for ct in range(NCT):
                    load_strip(xt_sb, b + 1, ct, nc.sync)
            yn_h = attention(b, qk_tiles, v_sb)
            cproj_phase(b, yn_h)

    nc.compile()
    return nc


# ---- host-side sharding / unsharding ----

def _rope_cos_sin():
    inv_freq = 1.0 / (ROPE_BASE ** (np.arange(0, D, 2, dtype=np.float32) / D))
    t = np.arange(T, dtype=np.float32)
    freqs = np.outer(t, inv_freq).astype(np.float32)
    emb = np.concatenate([freqs, freqs], axis=-1)
    return np.cos(emb).astype(np.float32), np.sin(emb).astype(np.float32)


def _tri():
    a = np.arange(128)[:, None]
    c = np.arange(128)[None, :]
    return (a <= c).astype(np.float32).astype(ml_dtypes.bfloat16)


_PROGRAM_CACHE = {}


def _get_program(with_bias_qk, with_bias_v):
    key = (with_bias_qk, with_bias_v)
    if key not in _PROGRAM_CACHE:
        _PROGRAM_CACHE[key] = _build_program(with_bias_qk, with_bias_v)
    return _PROGRAM_CACHE[key]


def _make_in_maps(x, W_attn, b_attn, W_proj):
    bf = ml_dtypes.bfloat16
    x = np.asarray(x, dtype=np.float32)
    W_attn = np.asarray(W_attn, dtype=np.float32)
    b_attn = np.asarray(b_attn, dtype=np.float32)
    W_proj = np.asarray(W_proj, dtype=np.float32)

    xT = np.ascontiguousarray(
        x.transpose(2, 0, 1).reshape(C, BT)
    ).astype(bf)
    Wq, Wk, Wv = W_attn[:, :C], W_attn[:, C : 2 * C], W_attn[:, 2 * C :]
    bq, bk, bvv = b_attn[:C], b_attn[C : 2 * C], b_attn[2 * C :]
    cos, sin = _rope_cos_sin()
    cosT = np.ascontiguousarray(cos.T)
    sinNT = np.ascontiguousarray(sin.T).copy()
    sinNT[:HD, :] *= -1.0  # sign-folded for the rotate_half DVE trick
    tri = _tri()

    in_maps = []
    for c in range(N_CORES):
        h0, h1 = HPC * c, HPC * c + 1
        sl0, sl1 = slice(h0 * D, (h0 + 1) * D), slice(h1 * D, (h1 + 1) * D)
        wqk_c = np.concatenate(
            [Wq[:, sl0], Wq[:, sl1], Wk[:, sl0], Wk[:, sl1]], axis=1
        ).astype(bf).reshape(NCT, 128, 4 * D).transpose(1, 0, 2)
        wv_c = (np.concatenate([Wv[:, sl0], Wv[:, sl1]], axis=1)
                .astype(bf).reshape(NCT, 128, HPC * D).transpose(1, 0, 2))
        wpr_c = (np.concatenate([W_proj[sl0, :], W_proj[sl1, :]], axis=0)
                 .astype(bf).reshape(HPC, 128, C).transpose(1, 0, 2))
        bqk_c = np.concatenate([bq[sl0], bq[sl1], bk[sl0], bk[sl1]]).astype(
            np.float32
        ).reshape(4, 128).T
        bv_c = np.concatenate([bvv[sl0], bvv[sl1]]).astype(np.float32)
        in_maps.append(
            {
                "xT": xT,
                "wqk": np.ascontiguousarray(wqk_c),
                "wv": np.ascontiguousarray(wv_c),
                "wpr": np.ascontiguousarray(wpr_c),
                "bqk": np.ascontiguousarray(bqk_c),
                "bqkr": np.ascontiguousarray(
                    np.concatenate([bqk_c[64:], bqk_c[:64]], axis=0)
                ),
                "bv": bv_c,
                "cosT": cosT,
                "sinNT": sinNT,
                "tri": tri,
            }
        )
    return in_maps


def _ensure_ntff_hook():
    """Bridge the missing antenv.axon_hooks module so trace=True can profile.

    The axon boot code registers an NTFF profiling hook via
    antenv.axon_hooks, which this image's antenv package lacks. Install a
    minimal in-memory module and register the ctypes-based hook from
    trn_agent_boot. Only used for profiling runs; best-effort.
    """
    import types

    if "antenv.axon_hooks" in sys.modules:
        return
    try:
        import antenv

        mod = types.ModuleType("antenv.axon_hooks")
        holder = {"hook": None}
        mod.set_axon_ntff_profile_hook = lambda h: holder.__setitem__("hook", h)
        mod.get_axon_ntff_profile_hook = lambda: holder["hook"]
        sys.modules["antenv.axon_hooks"] = mod
        antenv.axon_hooks = mod
        axon_site = "/root/.axon_site"
        if os.path.isdir(axon_site) and axon_site not in sys.path:
            sys.path.insert(0, axon_site)
        from trn_agent_boot.trn_boot import _ntff_profile_via_ctypes

        hook = _ntff_profile_via_ctypes("/opt/axon/libaxon_pjrt.so")
        if hook is not None:
            mod.set_axon_ntff_profile_hook(hook)
    except Exception as e:  # profiling is best-effort
        print(f"[ntff hook unavailable: {type(e).__name__}: {e}]", flush=True)


def run(x, W_attn, b_attn, W_proj, b_proj, trace=False):
    if trace:
        _ensure_ntff_hook()
        import concourse.bass_utils as _bu

        _bu.upload_artifacts = lambda tmpdir: f"local://{tmpdir}"
    b_attn = np.asarray(b_attn, dtype=np.float32)
    b_proj = np.asarray(b_proj, dtype=np.float32)
    with_bias_qk = bool(np.any(b_attn[: 2 * C] != 0.0))
    with_bias_v = bool(np.any(b_attn[2 * C :] != 0.0))
    nc = _get_program(with_bias_qk, with_bias_v)
    in_maps = _make_in_maps(x, W_attn, b_attn, W_proj)
    res = run_bass_kernel_spmd(
        nc, in_maps, list(range(N_CORES)), trace=trace
    )
    acc = np.zeros((BT, C), dtype=np.float32)
    for r in res.results:
        acc += np.asarray(r["out"], dtype=np.float32)
    acc += b_proj[None, :]
    return acc.reshape(B, T, C).astype(np.float32), res


def kernel(x, W_attn, b_attn, W_proj, b_proj):
    out, _ = run(x, W_attn, b_attn, W_proj, b_proj, trace=False)
    return out


# revision 38
# speedup vs baseline: 1.0157x; 1.0157x over previous
"""Trainium2 Bass kernel for causal self-attention with RoPE (tensor-parallel over 8 cores).

Contract: kernel(**inputs) takes full unsharded inputs (x, W_attn, b_attn,
W_proj, b_proj), shards across 8 NeuronCores (2 heads each), runs one SPMD
Bass/Tile kernel, and host-reduces the partial c_proj outputs.

Design notes (HW-measured best of 10 structural variants, ~433us vs 452us
baseline on core 0):
- RoPE entirely on DVE via partition-shifted reads of the chain psum with
  a sign-folded sin table (replaces 64 rotation matmuls + 32 scalar
  copies of the baseline).
- Causal column restriction: diagonal key-blocks only compute score/exp/
  attV/Z columns >= c0; a single shared [128,128] triangle mask handles
  the block-diagonal boundary (~15% less attention work than full-block).
- Softmax denominator Z accumulated on DVE (f32), finalized with a gpsimd
  partition_all_reduce (replaces 160 [1,512] PE matmuls, ~59us of PE).
- y-psum evacuated promptly via scalar copy (releases the bank without
  waiting on the DVE queue); 1/Z applied in a deferred DVE multiply.
- Diagonal-block attV/Z split so only the [128,128] masked window waits
  on the triangle-mask DVE op; the unmasked span proceeds immediately.
- Heads interleaved per q-tile with double-buffered y-PSUM.
- PSUM: qkv/v chains 2 banks, score pairs 2x[128,1024] 4 banks, y 2.
"""

import os
import sys

import numpy as np

for _p in ("/opt/trn_rl_repo",):
    if os.path.isdir(_p) and _p not in sys.path:
        sys.path.insert(0, _p)

import ml_dtypes
from contextlib import ExitStack

import concourse.bass as bass
import concourse.tile as tile
from concourse import bacc, bass_isa, mybir
from concourse.bass_utils import run_bass_kernel_spmd

# ---- problem constants (hardcoded per contract) ----
B, T, C = 2, 2048, 2048
H, D = 16, 128
N_CORES = 8
HPC = H // N_CORES  # heads per core = 2
ROPE_BASE = 10000.0
SCALE = float(1.0 / np.sqrt(D))
TQ = 512            # query tile (free dim of scores matmul)
NTQ = T // TQ       # 4
TK = 128            # key tile (partition dim of scoresT)
NTK = T // TK       # 16
NCT = C // 128      # 16 contraction tiles for projections
BT = B * T
HD = D // 2         # rope half

F32 = mybir.dt.float32
BF16 = mybir.dt.bfloat16

ADD = mybir.AluOpType.add
MULT = mybir.AluOpType.mult
EXP = mybir.ActivationFunctionType.Exp

PAIR_LOOKAHEAD = 2  # score-pairs ahead of attV in the attention pipeline


def _build_program(with_bias_qk: bool, with_bias_v: bool):
    nc = bacc.Bacc(
        "TRN2", target_bir_lowering=False, debug=False, num_devices=N_CORES
    )

    xT = nc.dram_tensor("xT", [C, BT], BF16, kind="ExternalInput").ap()
    wqk = nc.dram_tensor("wqk", [128, NCT, 4 * D], BF16, kind="ExternalInput").ap()
    wv = nc.dram_tensor("wv", [128, NCT, HPC * D], BF16, kind="ExternalInput").ap()
    wpr = nc.dram_tensor("wpr", [128, HPC, C], BF16, kind="ExternalInput").ap()
    bqk = nc.dram_tensor("bqk", [128, 4], F32, kind="ExternalInput").ap()
    bqkr = nc.dram_tensor("bqkr", [128, 4], F32, kind="ExternalInput").ap()
    bv = nc.dram_tensor("bv", [HPC * D], F32, kind="ExternalInput").ap()
    cosT = nc.dram_tensor("cosT", [D, T], F32, kind="ExternalInput").ap()
    sinNT = nc.dram_tensor("sinNT", [D, T], F32, kind="ExternalInput").ap()
    tri = nc.dram_tensor("tri", [128, 128], BF16, kind="ExternalInput").ap()
    out = nc.dram_tensor("out", [BT, C], BF16, kind="ExternalOutput").ap()

    with tile.TileContext(nc) as tc, ExitStack() as ctx:
        consts = ctx.enter_context(tc.tile_pool(name="consts", bufs=1))
        xt_pool = ctx.enter_context(tc.tile_pool(name="xt", bufs=1))
        qk_pool = ctx.enter_context(tc.tile_pool(name="qk", bufs=1))
        v_pool = ctx.enter_context(tc.tile_pool(name="v", bufs=1))
        e_pool = ctx.enter_context(tc.tile_pool(name="e", bufs=6))
        r_pool = ctx.enter_context(tc.tile_pool(name="rp", bufs=2))
        z_pool = ctx.enter_context(tc.tile_pool(name="zs", bufs=3))
        yn_pool = ctx.enter_context(tc.tile_pool(name="yn", bufs=1))
        ob_pool = ctx.enter_context(tc.tile_pool(name="ob", bufs=3))
        ps_mm = ctx.enter_context(tc.tile_pool(name="ps_mm", bufs=2, space="PSUM"))
        ps_s = ctx.enter_context(tc.tile_pool(name="ps_s", bufs=2, space="PSUM"))
        ps_y = ctx.enter_context(tc.tile_pool(name="ps_y", bufs=2, space="PSUM"))

        # ---- initial loads, interleaved in cold-start consumption order:
        # the cold loop eats (wqk[ct], strip[ct]) every ~0.9us, so small
        # leading wqk chunks + strips staggered across the 3 queues ----
        qs = [nc.sync, nc.gpsimd, nc.scalar]
        wqk_sb = consts.tile([128, NCT, 4 * D], BF16)

        def load_strip(xt_sb, b, ct, q):
            q.dma_start(
                xt_sb[:, ct, :],
                xT[ct * 128 : (ct + 1) * 128, b * T : (b + 1) * T],
            )

        xt_b0 = xt_pool.tile([128, NCT, T], BF16, tag="xt", name="xt_b0")
        nc.sync.dma_start(wqk_sb[:, 0:2, :], wqk[:, 0:2, :])
        nc.gpsimd.dma_start(wqk_sb[:, 2:5, :], wqk[:, 2:5, :])
        load_strip(xt_b0, 0, 2, nc.scalar)
        load_strip(xt_b0, 0, 0, nc.sync)
        load_strip(xt_b0, 0, 1, nc.gpsimd)
        load_strip(xt_b0, 0, 5, nc.scalar)
        nc.gpsimd.dma_start(wqk_sb[:, 5:9, :], wqk[:, 5:9, :])
        nc.scalar.dma_start(wqk_sb[:, 9:16, :], wqk[:, 9:16, :])
        for k, ct in enumerate((3, 6, 9, 12, 15)):
            load_strip(xt_b0, 0, ct, nc.sync)
        for ct in (4, 7, 10, 13):
            load_strip(xt_b0, 0, ct, nc.gpsimd)
        for ct in (8, 11, 14):
            load_strip(xt_b0, 0, ct, nc.scalar)

        cos_sb = consts.tile([128, T], F32)
        nc.sync.dma_start(cos_sb[:], cosT[:])
        sin_sb = consts.tile([128, T], F32)
        nc.gpsimd.dma_start(sin_sb[:], sinNT[:])
        tri_sb = consts.tile([128, 128], BF16)
        nc.scalar.dma_start(tri_sb[:], tri[:])
        wv_sb = consts.tile([128, NCT, HPC * D], BF16)
        nc.scalar.dma_start(wv_sb[:], wv[:])
        wpr_sb = consts.tile([128, HPC, C], BF16)
        nc.sync.dma_start(wpr_sb[:], wpr[:])
        if with_bias_qk:
            bqk_sb = consts.tile([128, 4], F32)
            nc.gpsimd.dma_start(bqk_sb[:], bqk[:])
        if with_bias_v:
            bv_sb = consts.tile([128, HPC * D], F32)
            nc.gpsimd.dma_start(bv_sb[:], bv.to_broadcast((128, HPC * D)))

        def emit_rope(f, t, w, ps, qk_tiles):
            """Matmul-free rope over w cols starting at q-tile t:
            qk[f][:, tsl] = (q+b)*cos + rot_half(q+b)*sinN.
            All four passes run on DVE; the shifted-base reads are legal
            because in0 is PSUM."""
            tsl = slice(t * TQ, t * TQ + w)
            b_all = bqk_sb[:, f : f + 1] if with_bias_qk else 0.0
            b_lo = bqk_sb[0:HD, f : f + 1] if with_bias_qk else 0.0
            b_hi = bqk_sb[HD:D, f : f + 1] if with_bias_qk else 0.0
            t1 = r_pool.tile([128, 2 * TQ], F32, tag="r1")
            nc.vector.scalar_tensor_tensor(
                t1[:, 0:w], ps[:, 0:w], b_all, cos_sb[:, tsl], op0=ADD, op1=MULT
            )
            t2 = r_pool.tile([128, 2 * TQ], F32, tag="r2")
            nc.vector.scalar_tensor_tensor(
                t2[0:HD, 0:w], ps[HD:D, 0:w], b_hi, sin_sb[0:HD, tsl],
                op0=ADD, op1=MULT,
            )
            nc.vector.scalar_tensor_tensor(
                t2[HD:D, 0:w], ps[0:HD, 0:w], b_lo, sin_sb[HD:D, tsl],
                op0=ADD, op1=MULT,
            )
            nc.vector.tensor_add(qk_tiles[f][:, tsl], t1[:, 0:w], t2[:, 0:w])

        def qkv_phase(b, xt_sb):
            """QKV projections + RoPE for batch b. Returns (qk_tiles, v_sb)."""
            # q/k feature tiles: 0=q_h0, 1=q_h1, 2=k_h0, 3=k_h1
            qk_tiles = [
                qk_pool.tile([128, T], BF16, tag=f"qk{f}", name=f"qkt{f}")
                for f in range(4)
            ]
            if b == 0:
                # cold start: t=0 for all four f-tiles ct-major so the PE
                # consumes xT strips as the initial DMAs land.
                cold_a = ps_s.tile([128, 2 * TQ], F32, tag="s", name="cold_a")
                cold_b = ps_s.tile([128, 2 * TQ], F32, tag="s", name="cold_b")
                t0_ps = [
                    cold_a[:, 0:TQ], cold_a[:, TQ : 2 * TQ],
                    cold_b[:, 0:TQ], cold_b[:, TQ : 2 * TQ],
                ]
                for ct in range(NCT):
                    for f in range(4):
                        nc.tensor.matmul(
                            t0_ps[f],
                            wqk_sb[:, ct, f * 128 : (f + 1) * 128],
                            xt_sb[:, ct, 0:TQ],
                            start=(ct == 0),
                            stop=(ct == NCT - 1),
                        )
                for f in range(4):
                    emit_rope(f, 0, TQ, t0_ps[f], qk_tiles)
            for f in range(4):
                for t in range(NTQ):
                    if b == 0 and t == 0:
                        continue
                    ps = ps_mm.tile([128, TQ], F32, tag="mm")
                    for ct in range(NCT):
                        nc.tensor.matmul(
                            ps[:],
                            wqk_sb[:, ct, f * 128 : (f + 1) * 128],
                            xt_sb[:, ct, t * TQ : (t + 1) * TQ],
                            start=(ct == 0),
                            stop=(ct == NCT - 1),
                        )
                    emit_rope(f, t, TQ, ps, qk_tiles)

            # V in [t, d] layout: lhsT = xT tile (c, t), rhs = Wv (c, d)
            v_sb = v_pool.tile([128, NTK, HPC * D], BF16, tag="v")
            for mt in range(NTK):
                ps = ps_mm.tile([128, HPC * D], F32, tag="mm")
                for ct in range(NCT):
                    nc.tensor.matmul(
                        ps[:],
                        xt_sb[:, ct, mt * 128 : (mt + 1) * 128],
                        wv_sb[:, ct, :],
                        start=(ct == 0),
                        stop=(ct == NCT - 1),
                    )
                if with_bias_v:
                    nc.vector.tensor_add(v_sb[:, mt, :], ps[:], bv_sb[:])
                else:
                    nc.scalar.copy(v_sb[:, mt, :], ps[:])
            return qk_tiles, v_sb

        def attention(b, qk_tiles, v_sb):
            """Flash-style causal attention, heads interleaved per q-tile.

            Returns yn tiles ([d, T] bf16, one per head)."""
            yn_h = [
                yn_pool.tile([128, T], BF16, tag=f"yn{hl}", name=f"yn{hl}")
                for hl in range(HPC)
            ]
            fin_backlog = []

            def emit_finalize(yps, zacc, hl, jsl):
                # evacuate yps promptly via scalar (its queue is right
                # behind this unit's exps) so the y-psum slot recycles
                # without waiting on the clogged DVE queue
                ysb = z_pool.tile([128, TQ], F32, tag="ysb", bufs=2)
                nc.scalar.copy(ysb[:], yps[:])
                zsum = z_pool.tile([128, TQ], F32, tag="zsum", bufs=2)
                nc.gpsimd.partition_all_reduce(
                    zsum[:], zacc[:], channels=128, reduce_op=bass_isa.ReduceOp.add
                )
                return (ysb, zsum, hl, jsl)

            def drain_finalize(ysb, zsum, hl, jsl):
                zrec = z_pool.tile([128, TQ], F32, tag="zrec", bufs=2)
                nc.vector.reciprocal_approx_fast(zrec[:], zsum[:])
                nc.vector.tensor_mul(yn_h[hl][:, jsl], ysb[:], zrec[:])

            for j in range(NTQ):
                jsl = slice(j * TQ, (j + 1) * TQ)
                nblk = 4 * j + 4
                # pairs of key-blocks: (i0, c0_of_i0, c0_of_i1); c0 = first
                # valid scores column (block-local) for causality.
                pairs = [(2 * p, 0, 0) for p in range(2 * j)]
                pairs.append((4 * j, 0, 128))
                pairs.append((4 * j + 2, 256, 384))
                npair = len(pairs)
                for hl in range(HPC):
                    qT = qk_tiles[hl]
                    kT = qk_tiles[2 + hl]
                    yps = ps_y.tile([128, TQ], F32, tag="y")
                    zacc = z_pool.tile([128, TQ], F32, tag="zacc")
                    e_tiles = [None] * npair

                    def emit_pair(p):
                        i0, c00, c01 = pairs[p]
                        sps = ps_s.tile([128, 2 * TQ], F32, tag="s")
                        for u, c0 in ((0, c00), (1, c01)):
                            i = i0 + u
                            nc.tensor.matmul(
                                sps[:, u * TQ + c0 : (u + 1) * TQ],
                                kT[:, i * TK : (i + 1) * TK],
                                qT[:, j * TQ + c0 : (j + 1) * TQ],
                                start=True,
                                stop=True,
                            )
                        e = e_pool.tile([128, 2 * TQ], BF16, tag="e")
                        # one exp over [c00 : 1024]; the gap columns
                        # [TQ : TQ+c01) hold garbage that is never read.
                        nc.scalar.activation(
                            e[:, c00:], sps[:, c00:], EXP, bias=0.0, scale=SCALE
                        )
                        e_tiles[p] = e

                    def emit_consume(p):
                        i0, c00, c01 = pairs[p]
                        e = e_tiles[p]
                        if c00 == 0 and c01 == 0:
                            # full pair: bf16 leaf sum (2x DVE rate), one
                            # f32 fold into zacc instead of two
                            zt = z_pool.tile([128, TQ], BF16, tag="zt",
                                             bufs=2)
                            nc.vector.tensor_add(
                                zt[:], e[:, 0:TQ], e[:, TQ : 2 * TQ]
                            )
                            if i0 == 0:
                                nc.vector.tensor_copy(zacc[:], zt[:])
                            else:
                                nc.vector.tensor_add(zacc[:], zacc[:], zt[:])
                            for u in range(2):
                                i = i0 + u
                                nc.tensor.matmul(
                                    yps[:],
                                    v_sb[:, i, hl * D : (hl + 1) * D],
                                    e[:, u * TQ : (u + 1) * TQ],
                                    start=(i == 0),
                                    stop=(i == nblk - 1),
                                )
                            return
                        for u, c0 in ((0, c00), (1, c01)):
                            i = i0 + u
                            vi = v_sb[:, i, hl * D : (hl + 1) * D]
                            first = i == 0
                            last = i == nblk - 1
                            if i < 4 * j or first:
                                # full block, or the group-opening block
                                # (j==0 r0): single start=True matmul so the
                                # psum init covers one contiguous region.
                                if first and i >= 4 * j:
                                    ew0 = e[:, u * TQ : u * TQ + 128]
                                    nc.vector.tensor_mul(ew0, ew0, tri_sb[:])
                                eh = e[:, u * TQ : (u + 1) * TQ]
                                if first:
                                    nc.vector.tensor_copy(zacc[:], eh)
                                else:
                                    nc.vector.tensor_add(zacc[:], zacc[:], eh)
                                nc.tensor.matmul(
                                    yps[:], vi, eh, start=first, stop=last
                                )
                                continue
                            # diagonal block: the unmasked span [c0+128:TQ]
                            # proceeds without waiting on the mask; only the
                            # [128,128] masked window is gated on DVE.
                            whi = c0 + 128
                            if whi < TQ:
                                ehb = e[:, u * TQ + whi : (u + 1) * TQ]
                                nc.vector.tensor_add(
                                    zacc[:, whi:], zacc[:, whi:], ehb
                                )
                                nc.tensor.matmul(
                                    yps[:, whi:], vi, ehb,
                                    start=False, stop=False,
                                )
                            ew = e[:, u * TQ + c0 : u * TQ + whi]
                            nc.vector.tensor_mul(ew, ew, tri_sb[:])
                            nc.vector.tensor_add(
                                zacc[:, c0:whi], zacc[:, c0:whi], ew
                            )
                            nc.tensor.matmul(
                                yps[:, c0:whi], vi, ew, start=False, stop=last
                            )

                    for p in range(npair):
                        emit_pair(p)
                        if p >= PAIR_LOOKAHEAD:
                            emit_consume(p - PAIR_LOOKAHEAD)
                    for p in range(max(0, npair - PAIR_LOOKAHEAD), npair):
                        emit_consume(p)

                    fin_backlog.append(emit_finalize(yps, zacc, hl, jsl))
                    # drain the previous (j,hl)'s finalize now: its gpsimd
                    # all-reduce has had a full head-slot to complete, so the
                    # DVE queue won't stall on it.
                    if len(fin_backlog) > 1:
                        drain_finalize(*fin_backlog.pop(0))
            while fin_backlog:
                drain_finalize(*fin_backlog.pop(0))
            return yn_h

        def cproj_phase(b, yn_h):
            oq = [nc.sync, nc.gpsimd]
            for mt in range(NTK):
                osb = ob_pool.tile([128, C], BF16, tag="ob")
                for np_ in range(NTQ // 2):
                    ops = ps_s.tile([128, 2 * TQ], F32, tag="s")
                    for u in range(2):
                        n = 2 * np_ + u
                        nsl_ps = slice(u * TQ, (u + 1) * TQ)
                        for hl in range(HPC):
                            nc.tensor.matmul(
                                ops[:, nsl_ps],
                                yn_h[hl][:, mt * 128 : (mt + 1) * 128],
                                wpr_sb[:, hl, n * TQ : (n + 1) * TQ],
                                start=(hl == 0),
                                stop=(hl == HPC - 1),
                            )
                    osl = slice(2 * np_ * TQ, 2 * (np_ + 1) * TQ)
                    if np_ % 2 == 0:
                        nc.vector.tensor_copy(osb[:, osl], ops[:])
                    else:
                        nc.scalar.copy(osb[:, osl], ops[:])
                oq[mt % 2].dma_start(
                    out[b * T + mt * 128 : b * T + (mt + 1) * 128, :], osb[:]
                )

        xt_sb = xt_b0
        for b in range(B):
            qk_tiles, v_sb = qkv_phase(b, xt_sb)
            if b + 1 < B:
                xt_sb = xt_pool.tile([128, NCT, T], BF16, tag="xt", name="xt_b1")
                for ct in range(NCT):
                    load_strip(xt_sb, b + 1, ct, nc.sync)
            yn_h = attention(b, qk_tiles, v_sb)
            cproj_phase(b, yn_h)

    nc.compile()
    return nc


# ---- host-side sharding / unsharding ----

def _rope_cos_sin():
    inv_freq = 1.0 / (ROPE_BASE ** (np.arange(0, D, 2, dtype=np.float32) / D))
    t = np.arange(T, dtype=np.float32)
    freqs = np.outer(t, inv_freq).astype(np.float32)
    emb = np.concatenate([freqs, freqs], axis=-1)
    return np.cos(emb).astype(np.float32), np.sin(emb).astype(np.float32)


def _tri():
    a = np.arange(128)[:, None]
    c = np.arange(128)[None, :]
    return (a <= c).astype(np.float32).astype(ml_dtypes.bfloat16)


_PROGRAM_CACHE = {}


def _get_program(with_bias_qk, with_bias_v):
    key = (with_bias_qk, with_bias_v)
    if key not in _PROGRAM_CACHE:
        _PROGRAM_CACHE[key] = _build_program(with_bias_qk, with_bias_v)
    return _PROGRAM_CACHE[key]


def _make_in_maps(x, W_attn, b_attn, W_proj):
    bf = ml_dtypes.bfloat16
    x = np.asarray(x, dtype=np.float32)
    W_attn = np.asarray(W_attn, dtype=np.float32)
    b_attn = np.asarray(b_attn, dtype=np.float32)
    W_proj = np.asarray(W_proj, dtype=np.float32)

    xT = np.ascontiguousarray(
        x.transpose(2, 0, 1).reshape(C, BT)
    ).astype(bf)
    Wq, Wk, Wv = W_attn[:, :C], W_attn[:, C : 2 * C], W_attn[:, 2 * C :]
    bq, bk, bvv = b_attn[:C], b_attn[C : 2 * C], b_attn[2 * C :]
    cos, sin = _rope_cos_sin()
    cosT = np.ascontiguousarray(cos.T)
    sinNT = np.ascontiguousarray(sin.T).copy()
    sinNT[:HD, :] *= -1.0  # sign-folded for the rotate_half DVE trick
    tri = _tri()

    in_maps = []
    for c in range(N_CORES):
        h0, h1 = HPC * c, HPC * c + 1
        sl0, sl1 = slice(h0 * D, (h0 + 1) * D), slice(h1 * D, (h1 + 1) * D)
        wqk_c = np.concatenate(
            [Wq[:, sl0], Wq[:, sl1], Wk[:, sl0], Wk[:, sl1]], axis=1
        ).astype(bf).reshape(NCT, 128, 4 * D).transpose(1, 0, 2)
        wv_c = (np.concatenate([Wv[:, sl0], Wv[:, sl1]], axis=1)
                .astype(bf).reshape(NCT, 128, HPC * D).transpose(1, 0, 2))
        wpr_c = (np.concatenate([W_proj[sl0, :], W_proj[sl1, :]], axis=0)
                 .astype(bf).reshape(HPC, 128, C).transpose(1, 0, 2))
        bqk_c = np.concatenate([bq[sl0], bq[sl1], bk[sl0], bk[sl1]]).astype(
            np.float32
        ).reshape(4, 128).T
        bv_c = np.concatenate([bvv[sl0], bvv[sl1]]).astype(np.float32)
        in_maps.append(
            {
                "xT": xT,
                "wqk": np.ascontiguousarray(wqk_c),
                "wv": np.ascontiguousarray(wv_c),
                "wpr": np.ascontiguousarray(wpr_c),
                "bqk": np.ascontiguousarray(bqk_c),
                "bqkr": np.ascontiguousarray(
                    np.concatenate([bqk_c[64:], bqk_c[:64]], axis=0)
                ),
                "bv": bv_c,
                "cosT": cosT,
                "sinNT": sinNT,
                "tri": tri,
            }
        )
    return in_maps


def _ensure_ntff_hook():
    """Bridge the missing antenv.axon_hooks module so trace=True can profile.

    The axon boot code registers an NTFF profiling hook via
    antenv.axon_hooks, which this image's antenv package lacks. Install a
    minimal in-memory module and register the ctypes-based hook from
    trn_agent_boot. Only used for profiling runs; best-effort.
    """
    import types

    if "antenv.axon_hooks" in sys.modules:
        return
    try:
        import antenv

        mod = types.ModuleType("antenv.axon_hooks")
        holder = {"hook": None}
        mod.set_axon_ntff_profile_hook = lambda h: holder.__setitem__("hook", h)
        mod.get_axon_ntff_profile_hook = lambda: holder["hook"]
        sys.modules["antenv.axon_hooks"] = mod
        antenv.axon_hooks = mod
        axon_site = "/root/.axon_site"
        if os.path.isdir(axon_site) and axon_site not in sys.path:
            sys.path.insert(0, axon_site)
        from trn_agent_boot.trn_boot import _ntff_profile_via_ctypes

        hook = _ntff_profile_via_ctypes("/opt/axon/libaxon_pjrt.so")
        if hook is not None:
            mod.set_axon_ntff_profile_hook(hook)
    except Exception as e:  # profiling is best-effort
        print(f"[ntff hook unavailable: {type(e).__name__}: {e}]", flush=True)


def run(x, W_attn, b_attn, W_proj, b_proj, trace=False):
    if trace:
        _ensure_ntff_hook()
        import concourse.bass_utils as _bu

        _bu.upload_artifacts = lambda tmpdir: f"local://{tmpdir}"
    b_attn = np.asarray(b_attn, dtype=np.float32)
    b_proj = np.asarray(b_proj, dtype=np.float32)
    with_bias_qk = bool(np.any(b_attn[: 2 * C] != 0.0))
    with_bias_v = bool(np.any(b_attn[2 * C :] != 0.0))
    nc = _get_program(with_bias_qk, with_bias_v)
    in_maps = _make_in_maps(x, W_attn, b_attn, W_proj)
    res = run_bass_kernel_spmd(
        nc, in_maps, list(range(N_CORES)), trace=trace
    )
    acc = np.zeros((BT, C), dtype=np.float32)
    for r in res.results:
        acc += np.asarray(r["out"], dtype=np.float32)
    acc += b_proj[None, :]
    return acc.reshape(B, T, C).astype(np.float32), res


def kernel(x, W_attn, b_attn, W_proj, b_proj):
    out, _ = run(x, W_attn, b_attn, W_proj, b_proj, trace=False)
    return out


# revision 39
# speedup vs baseline: 1.0288x; 1.0128x over previous
"""Trainium2 Bass kernel for causal self-attention with RoPE (tensor-parallel over 8 cores).

Contract: kernel(**inputs) takes full unsharded inputs (x, W_attn, b_attn,
W_proj, b_proj), shards across 8 NeuronCores (2 heads each), runs one SPMD
Bass/Tile kernel, and host-reduces the partial c_proj outputs.

Design notes (HW-measured best of 10 structural variants, ~433us vs 452us
baseline on core 0):
- RoPE entirely on DVE via partition-shifted reads of the chain psum with
  a sign-folded sin table (replaces 64 rotation matmuls + 32 scalar
  copies of the baseline).
- Causal column restriction: diagonal key-blocks only compute score/exp/
  attV/Z columns >= c0; a single shared [128,128] triangle mask handles
  the block-diagonal boundary (~15% less attention work than full-block).
- Softmax denominator Z accumulated on DVE (f32), finalized with a gpsimd
  partition_all_reduce (replaces 160 [1,512] PE matmuls, ~59us of PE).
- y-psum evacuated promptly via scalar copy (releases the bank without
  waiting on the DVE queue); 1/Z applied in a deferred DVE multiply.
- Diagonal-block attV/Z split so only the [128,128] masked window waits
  on the triangle-mask DVE op; the unmasked span proceeds immediately.
- Heads interleaved per q-tile with double-buffered y-PSUM.
- PSUM: qkv/v chains 2 banks, score pairs 2x[128,1024] 4 banks, y 2.
"""

import os
import sys

import numpy as np

for _p in ("/opt/trn_rl_repo",):
    if os.path.isdir(_p) and _p not in sys.path:
        sys.path.insert(0, _p)

import ml_dtypes
from contextlib import ExitStack

import concourse.bass as bass
import concourse.tile as tile
from concourse import bacc, bass_isa, mybir
from concourse.bass_utils import run_bass_kernel_spmd

# ---- problem constants (hardcoded per contract) ----
B, T, C = 2, 2048, 2048
H, D = 16, 128
N_CORES = 8
HPC = H // N_CORES  # heads per core = 2
ROPE_BASE = 10000.0
SCALE = float(1.0 / np.sqrt(D))
TQ = 512            # query tile (free dim of scores matmul)
NTQ = T // TQ       # 4
TK = 128            # key tile (partition dim of scoresT)
NTK = T // TK       # 16
NCT = C // 128      # 16 contraction tiles for projections
BT = B * T
HD = D // 2         # rope half

F32 = mybir.dt.float32
BF16 = mybir.dt.bfloat16

ADD = mybir.AluOpType.add
MULT = mybir.AluOpType.mult
EXP = mybir.ActivationFunctionType.Exp

PAIR_LOOKAHEAD = 2  # score-pairs ahead of attV in the attention pipeline


def _build_program(with_bias_qk: bool, with_bias_v: bool):
    nc = bacc.Bacc(
        "TRN2", target_bir_lowering=False, debug=False, num_devices=N_CORES
    )

    xT = nc.dram_tensor("xT", [C, BT], BF16, kind="ExternalInput").ap()
    wqk = nc.dram_tensor("wqk", [128, NCT, 4 * D], BF16, kind="ExternalInput").ap()
    wv = nc.dram_tensor("wv", [128, NCT, HPC * D], BF16, kind="ExternalInput").ap()
    wpr = nc.dram_tensor("wpr", [128, HPC, C], BF16, kind="ExternalInput").ap()
    bqk = nc.dram_tensor("bqk", [128, 4], F32, kind="ExternalInput").ap()
    bqkr = nc.dram_tensor("bqkr", [128, 4], F32, kind="ExternalInput").ap()
    bv = nc.dram_tensor("bv", [HPC * D], F32, kind="ExternalInput").ap()
    cosT = nc.dram_tensor("cosT", [D, T], F32, kind="ExternalInput").ap()
    sinNT = nc.dram_tensor("sinNT", [D, T], F32, kind="ExternalInput").ap()
    tri = nc.dram_tensor("tri", [128, 128], BF16, kind="ExternalInput").ap()
    out = nc.dram_tensor("out", [BT, C], BF16, kind="ExternalOutput").ap()

    with tile.TileContext(nc) as tc, ExitStack() as ctx:
        consts = ctx.enter_context(tc.tile_pool(name="consts", bufs=1))
        xt_pool = ctx.enter_context(tc.tile_pool(name="xt", bufs=1))
        qk_pool = ctx.enter_context(tc.tile_pool(name="qk", bufs=1))
        v_pool = ctx.enter_context(tc.tile_pool(name="v", bufs=1))
        e_pool = ctx.enter_context(tc.tile_pool(name="e", bufs=8))
        r_pool = ctx.enter_context(tc.tile_pool(name="rp", bufs=2))
        z_pool = ctx.enter_context(tc.tile_pool(name="zs", bufs=3))
        yn_pool = ctx.enter_context(tc.tile_pool(name="yn", bufs=1))
        ob_pool = ctx.enter_context(tc.tile_pool(name="ob", bufs=3))
        ps_mm = ctx.enter_context(tc.tile_pool(name="ps_mm", bufs=2, space="PSUM"))
        ps_s = ctx.enter_context(tc.tile_pool(name="ps_s", bufs=2, space="PSUM"))
        ps_y = ctx.enter_context(tc.tile_pool(name="ps_y", bufs=2, space="PSUM"))

        # ---- initial loads, interleaved in cold-start consumption order:
        # the cold loop eats (wqk[ct], strip[ct]) every ~0.9us, so small
        # leading wqk chunks + strips staggered across the 3 queues ----
        qs = [nc.sync, nc.gpsimd, nc.scalar]
        wqk_sb = consts.tile([128, NCT, 4 * D], BF16)

        def load_strip(xt_sb, b, ct, q):
            q.dma_start(
                xt_sb[:, ct, :],
                xT[ct * 128 : (ct + 1) * 128, b * T : (b + 1) * T],
            )

        xt_b0 = xt_pool.tile([128, NCT, T], BF16, tag="xt", name="xt_b0")
        nc.sync.dma_start(wqk_sb[:, 0:2, :], wqk[:, 0:2, :])
        nc.gpsimd.dma_start(wqk_sb[:, 2:5, :], wqk[:, 2:5, :])
        load_strip(xt_b0, 0, 2, nc.scalar)
        load_strip(xt_b0, 0, 0, nc.sync)
        load_strip(xt_b0, 0, 1, nc.gpsimd)
        load_strip(xt_b0, 0, 5, nc.scalar)
        nc.gpsimd.dma_start(wqk_sb[:, 5:9, :], wqk[:, 5:9, :])
        nc.scalar.dma_start(wqk_sb[:, 9:16, :], wqk[:, 9:16, :])
        for k, ct in enumerate((3, 6, 9, 12, 15)):
            load_strip(xt_b0, 0, ct, nc.sync)
        for ct in (4, 7, 10, 13):
            load_strip(xt_b0, 0, ct, nc.gpsimd)
        for ct in (8, 11, 14):
            load_strip(xt_b0, 0, ct, nc.scalar)

        cos_sb = consts.tile([128, T], F32)
        nc.sync.dma_start(cos_sb[:], cosT[:])
        sin_sb = consts.tile([128, T], F32)
        nc.gpsimd.dma_start(sin_sb[:], sinNT[:])
        tri_sb = consts.tile([128, 128], BF16)
        nc.scalar.dma_start(tri_sb[:], tri[:])
        wv_sb = consts.tile([128, NCT, HPC * D], BF16)
        nc.scalar.dma_start(wv_sb[:], wv[:])
        wpr_sb = consts.tile([128, HPC, C], BF16)
        nc.sync.dma_start(wpr_sb[:], wpr[:])
        if with_bias_qk:
            bqk_sb = consts.tile([128, 4], F32)
            nc.gpsimd.dma_start(bqk_sb[:], bqk[:])
        if with_bias_v:
            bv_sb = consts.tile([128, HPC * D], F32)
            nc.gpsimd.dma_start(bv_sb[:], bv.to_broadcast((128, HPC * D)))

        def emit_rope(f, t, w, ps, qk_tiles):
            """Matmul-free rope over w cols starting at q-tile t:
            qk[f][:, tsl] = (q+b)*cos + rot_half(q+b)*sinN.
            All four passes run on DVE; the shifted-base reads are legal
            because in0 is PSUM."""
            tsl = slice(t * TQ, t * TQ + w)
            b_all = bqk_sb[:, f : f + 1] if with_bias_qk else 0.0
            b_lo = bqk_sb[0:HD, f : f + 1] if with_bias_qk else 0.0
            b_hi = bqk_sb[HD:D, f : f + 1] if with_bias_qk else 0.0
            t1 = r_pool.tile([128, 2 * TQ], BF16, tag="r1", bufs=3)
            nc.vector.scalar_tensor_tensor(
                t1[:, 0:w], ps[:, 0:w], b_all, cos_sb[:, tsl], op0=ADD, op1=MULT
            )
            t2 = r_pool.tile([128, 2 * TQ], BF16, tag="r2", bufs=3)
            nc.vector.scalar_tensor_tensor(
                t2[0:HD, 0:w], ps[HD:D, 0:w], b_hi, sin_sb[0:HD, tsl],
                op0=ADD, op1=MULT,
            )
            nc.vector.scalar_tensor_tensor(
                t2[HD:D, 0:w], ps[0:HD, 0:w], b_lo, sin_sb[HD:D, tsl],
                op0=ADD, op1=MULT,
            )
            nc.vector.tensor_add(qk_tiles[f][:, tsl], t1[:, 0:w], t2[:, 0:w])

        def qkv_phase(b, xt_sb):
            """QKV projections + RoPE for batch b. Returns (qk_tiles, v_sb)."""
            # q/k feature tiles: 0=q_h0, 1=q_h1, 2=k_h0, 3=k_h1
            qk_tiles = [
                qk_pool.tile([128, T], BF16, tag=f"qk{f}", name=f"qkt{f}")
                for f in range(4)
            ]
            if b == 0:
                # cold start: t=0 for all four f-tiles ct-major so the PE
                # consumes xT strips as the initial DMAs land.
                cold_a = ps_s.tile([128, 2 * TQ], F32, tag="s", name="cold_a")
                cold_b = ps_s.tile([128, 2 * TQ], F32, tag="s", name="cold_b")
                t0_ps = [
                    cold_a[:, 0:TQ], cold_a[:, TQ : 2 * TQ],
                    cold_b[:, 0:TQ], cold_b[:, TQ : 2 * TQ],
                ]
                for ct in range(NCT):
                    for f in range(4):
                        nc.tensor.matmul(
                            t0_ps[f],
                            wqk_sb[:, ct, f * 128 : (f + 1) * 128],
                            xt_sb[:, ct, 0:TQ],
                            start=(ct == 0),
                            stop=(ct == NCT - 1),
                        )
                for f in range(4):
                    emit_rope(f, 0, TQ, t0_ps[f], qk_tiles)
            for f in range(4):
                for t in range(NTQ):
                    if b == 0 and t == 0:
                        continue
                    ps = ps_mm.tile([128, TQ], F32, tag="mm")
                    for ct in range(NCT):
                        nc.tensor.matmul(
                            ps[:],
                            wqk_sb[:, ct, f * 128 : (f + 1) * 128],
                            xt_sb[:, ct, t * TQ : (t + 1) * TQ],
                            start=(ct == 0),
                            stop=(ct == NCT - 1),
                        )
                    emit_rope(f, t, TQ, ps, qk_tiles)

            # V in [t, d] layout: lhsT = xT tile (c, t), rhs = Wv (c, d)
            v_sb = v_pool.tile([128, NTK, HPC * D], BF16, tag="v")
            for mt in range(NTK):
                ps = ps_mm.tile([128, HPC * D], F32, tag="mm")
                for ct in range(NCT):
                    nc.tensor.matmul(
                        ps[:],
                        xt_sb[:, ct, mt * 128 : (mt + 1) * 128],
                        wv_sb[:, ct, :],
                        start=(ct == 0),
                        stop=(ct == NCT - 1),
                    )
                if with_bias_v:
                    nc.vector.tensor_add(v_sb[:, mt, :], ps[:], bv_sb[:])
                else:
                    nc.scalar.copy(v_sb[:, mt, :], ps[:])
            return qk_tiles, v_sb

        def attention(b, qk_tiles, v_sb):
            """Flash-style causal attention, heads interleaved per q-tile.

            Returns yn tiles ([d, T] bf16, one per head)."""
            yn_h = [
                yn_pool.tile([128, T], BF16, tag=f"yn{hl}", name=f"yn{hl}")
                for hl in range(HPC)
            ]
            fin_backlog = []

            def emit_finalize(yps, zacc, hl, jsl):
                # evacuate yps promptly via scalar (its queue is right
                # behind this unit's exps) so the y-psum slot recycles
                # without waiting on the clogged DVE queue
                ysb = z_pool.tile([128, TQ], F32, tag="ysb", bufs=2)
                nc.scalar.copy(ysb[:], yps[:])
                zsum = z_pool.tile([128, TQ], F32, tag="zsum", bufs=2)
                nc.gpsimd.partition_all_reduce(
                    zsum[:], zacc[:], channels=128, reduce_op=bass_isa.ReduceOp.add
                )
                return (ysb, zsum, hl, jsl)

            def drain_finalize(ysb, zsum, hl, jsl):
                zrec = z_pool.tile([128, TQ], F32, tag="zrec", bufs=2)
                nc.vector.reciprocal_approx_fast(zrec[:], zsum[:])
                nc.vector.tensor_mul(yn_h[hl][:, jsl], ysb[:], zrec[:])

            for j in range(NTQ):
                jsl = slice(j * TQ, (j + 1) * TQ)
                nblk = 4 * j + 4
                # pairs of key-blocks: (i0, c0_of_i0, c0_of_i1); c0 = first
                # valid scores column (block-local) for causality.
                pairs = [(2 * p, 0, 0) for p in range(2 * j)]
                pairs.append((4 * j, 0, 128))
                pairs.append((4 * j + 2, 256, 384))
                npair = len(pairs)
                for hl in range(HPC):
                    qT = qk_tiles[hl]
                    kT = qk_tiles[2 + hl]
                    yps = ps_y.tile([128, TQ], F32, tag="y")
                    zacc = z_pool.tile([128, TQ], F32, tag="zacc")
                    e_tiles = [None] * npair

                    def emit_pair(p):
                        i0, c00, c01 = pairs[p]
                        sps = ps_s.tile([128, 2 * TQ], F32, tag="s")
                        for u, c0 in ((0, c00), (1, c01)):
                            i = i0 + u
                            nc.tensor.matmul(
                                sps[:, u * TQ + c0 : (u + 1) * TQ],
                                kT[:, i * TK : (i + 1) * TK],
                                qT[:, j * TQ + c0 : (j + 1) * TQ],
                                start=True,
                                stop=True,
                            )
                        e = e_pool.tile([128, 2 * TQ], BF16, tag="e")
                        # one exp over [c00 : 1024]; the gap columns
                        # [TQ : TQ+c01) hold garbage that is never read.
                        nc.scalar.activation(
                            e[:, c00:], sps[:, c00:], EXP, bias=0.0, scale=SCALE
                        )
                        e_tiles[p] = e

                    def emit_consume(p):
                        i0, c00, c01 = pairs[p]
                        e = e_tiles[p]
                        if c00 == 0 and c01 == 0:
                            # full pair: bf16 leaf sum (2x DVE rate), one
                            # f32 fold into zacc instead of two
                            zt = z_pool.tile([128, TQ], BF16, tag="zt",
                                             bufs=2)
                            nc.vector.tensor_add(
                                zt[:], e[:, 0:TQ], e[:, TQ : 2 * TQ]
                            )
                            if i0 == 0:
                                nc.vector.tensor_copy(zacc[:], zt[:])
                            else:
                                nc.vector.tensor_add(zacc[:], zacc[:], zt[:])
                            for u in range(2):
                                i = i0 + u
                                nc.tensor.matmul(
                                    yps[:],
                                    v_sb[:, i, hl * D : (hl + 1) * D],
                                    e[:, u * TQ : (u + 1) * TQ],
                                    start=(i == 0),
                                    stop=(i == nblk - 1),
                                )
                            return
                        for u, c0 in ((0, c00), (1, c01)):
                            i = i0 + u
                            vi = v_sb[:, i, hl * D : (hl + 1) * D]
                            first = i == 0
                            last = i == nblk - 1
                            if i < 4 * j or first:
                                # full block, or the group-opening block
                                # (j==0 r0): single start=True matmul so the
                                # psum init covers one contiguous region.
                                if first and i >= 4 * j:
                                    ew0 = e[:, u * TQ : u * TQ + 128]
                                    nc.vector.tensor_mul(ew0, ew0, tri_sb[:])
                                eh = e[:, u * TQ : (u + 1) * TQ]
                                if first:
                                    nc.vector.tensor_copy(zacc[:], eh)
                                else:
                                    nc.vector.tensor_add(zacc[:], zacc[:], eh)
                                nc.tensor.matmul(
                                    yps[:], vi, eh, start=first, stop=last
                                )
                                continue
                            # diagonal block: the unmasked span [c0+128:TQ]
                            # proceeds without waiting on the mask; only the
                            # [128,128] masked window is gated on DVE.
                            whi = c0 + 128
                            if whi < TQ:
                                ehb = e[:, u * TQ + whi : (u + 1) * TQ]
                                nc.vector.tensor_add(
                                    zacc[:, whi:], zacc[:, whi:], ehb
                                )
                                nc.tensor.matmul(
                                    yps[:, whi:], vi, ehb,
                                    start=False, stop=False,
                                )
                            ew = e[:, u * TQ + c0 : u * TQ + whi]
                            nc.vector.tensor_mul(ew, ew, tri_sb[:])
                            nc.vector.tensor_add(
                                zacc[:, c0:whi], zacc[:, c0:whi], ew
                            )
                            nc.tensor.matmul(
                                yps[:, c0:whi], vi, ew, start=False, stop=last
                            )

                    for p in range(npair):
                        emit_pair(p)
                        if p >= PAIR_LOOKAHEAD:
                            emit_consume(p - PAIR_LOOKAHEAD)
                    for p in range(max(0, npair - PAIR_LOOKAHEAD), npair):
                        emit_consume(p)

                    fin_backlog.append(emit_finalize(yps, zacc, hl, jsl))
                    # drain the previous (j,hl)'s finalize now: its gpsimd
                    # all-reduce has had a full head-slot to complete, so the
                    # DVE queue won't stall on it.
                    if len(fin_backlog) > 1:
                        drain_finalize(*fin_backlog.pop(0))
            while fin_backlog:
                drain_finalize(*fin_backlog.pop(0))
            return yn_h

        def cproj_phase(b, yn_h):
            oq = [nc.sync, nc.gpsimd]
            for mt in range(NTK):
                osb = ob_pool.tile([128, C], BF16, tag="ob")
                for np_ in range(NTQ // 2):
                    ops = ps_s.tile([128, 2 * TQ], F32, tag="s")
                    for u in range(2):
                        n = 2 * np_ + u
                        nsl_ps = slice(u * TQ, (u + 1) * TQ)
                        for hl in range(HPC):
                            nc.tensor.matmul(
                                ops[:, nsl_ps],
                                yn_h[hl][:, mt * 128 : (mt + 1) * 128],
                                wpr_sb[:, hl, n * TQ : (n + 1) * TQ],
                                start=(hl == 0),
                                stop=(hl == HPC - 1),
                            )
                    osl = slice(2 * np_ * TQ, 2 * (np_ + 1) * TQ)
                    if np_ % 2 == 0:
                        nc.vector.tensor_copy(osb[:, osl], ops[:])
                    else:
                        nc.scalar.copy(osb[:, osl], ops[:])
                oq[mt % 2].dma_start(
                    out[b * T + mt * 128 : b * T + (mt + 1) * 128, :], osb[:]
                )

        xt_sb = xt_b0
        for b in range(B):
            qk_tiles, v_sb = qkv_phase(b, xt_sb)
            if b + 1 < B:
                xt_sb = xt_pool.tile([128, NCT, T], BF16, tag="xt", name="xt_b1")
                for ct in range(NCT):
                    load_strip(xt_sb, b + 1, ct, nc.sync)
            yn_h = attention(b, qk_tiles, v_sb)
            cproj_phase(b, yn_h)

    nc.compile()
    return nc


# ---- host-side sharding / unsharding ----

def _rope_cos_sin():
    inv_freq = 1.0 / (ROPE_BASE ** (np.arange(0, D, 2, dtype=np.float32) / D))
    t = np.arange(T, dtype=np.float32)
    freqs = np.outer(t, inv_freq).astype(np.float32)
    emb = np.concatenate([freqs, freqs], axis=-1)
    return np.cos(emb).astype(np.float32), np.sin(emb).astype(np.float32)


def _tri():
    a = np.arange(128)[:, None]
    c = np.arange(128)[None, :]
    return (a <= c).astype(np.float32).astype(ml_dtypes.bfloat16)


_PROGRAM_CACHE = {}


def _get_program(with_bias_qk, with_bias_v):
    key = (with_bias_qk, with_bias_v)
    if key not in _PROGRAM_CACHE:
        _PROGRAM_CACHE[key] = _build_program(with_bias_qk, with_bias_v)
    return _PROGRAM_CACHE[key]


def _make_in_maps(x, W_attn, b_attn, W_proj):
    bf = ml_dtypes.bfloat16
    x = np.asarray(x, dtype=np.float32)
    W_attn = np.asarray(W_attn, dtype=np.float32)
    b_attn = np.asarray(b_attn, dtype=np.float32)
    W_proj = np.asarray(W_proj, dtype=np.float32)

    xT = np.ascontiguousarray(
        x.transpose(2, 0, 1).reshape(C, BT)
    ).astype(bf)
    Wq, Wk, Wv = W_attn[:, :C], W_attn[:, C : 2 * C], W_attn[:, 2 * C :]
    bq, bk, bvv = b_attn[:C], b_attn[C : 2 * C], b_attn[2 * C :]
    cos, sin = _rope_cos_sin()
    cosT = np.ascontiguousarray(cos.T)
    sinNT = np.ascontiguousarray(sin.T).copy()
    sinNT[:HD, :] *= -1.0  # sign-folded for the rotate_half DVE trick
    tri = _tri()

    in_maps = []
    for c in range(N_CORES):
        h0, h1 = HPC * c, HPC * c + 1
        sl0, sl1 = slice(h0 * D, (h0 + 1) * D), slice(h1 * D, (h1 + 1) * D)
        wqk_c = np.concatenate(
            [Wq[:, sl0], Wq[:, sl1], Wk[:, sl0], Wk[:, sl1]], axis=1
        ).astype(bf).reshape(NCT, 128, 4 * D).transpose(1, 0, 2)
        wv_c = (np.concatenate([Wv[:, sl0], Wv[:, sl1]], axis=1)
                .astype(bf).reshape(NCT, 128, HPC * D).transpose(1, 0, 2))
        wpr_c = (np.concatenate([W_proj[sl0, :], W_proj[sl1, :]], axis=0)
                 .astype(bf).reshape(HPC, 128, C).transpose(1, 0, 2))
        bqk_c = np.concatenate([bq[sl0], bq[sl1], bk[sl0], bk[sl1]]).astype(
            np.float32
        ).reshape(4, 128).T
        bv_c = np.concatenate([bvv[sl0], bvv[sl1]]).astype(np.float32)
        in_maps.append(
            {
                "xT": xT,
                "wqk": np.ascontiguousarray(wqk_c),
                "wv": np.ascontiguousarray(wv_c),
                "wpr": np.ascontiguousarray(wpr_c),
                "bqk": np.ascontiguousarray(bqk_c),
                "bqkr": np.ascontiguousarray(
                    np.concatenate([bqk_c[64:], bqk_c[:64]], axis=0)
                ),
                "bv": bv_c,
                "cosT": cosT,
                "sinNT": sinNT,
                "tri": tri,
            }
        )
    return in_maps


def _ensure_ntff_hook():
    """Bridge the missing antenv.axon_hooks module so trace=True can profile.

    The axon boot code registers an NTFF profiling hook via
    antenv.axon_hooks, which this image's antenv package lacks. Install a
    minimal in-memory module and register the ctypes-based hook from
    trn_agent_boot. Only used for profiling runs; best-effort.
    """
    import types

    if "antenv.axon_hooks" in sys.modules:
        return
    try:
        import antenv

        mod = types.ModuleType("antenv.axon_hooks")
        holder = {"hook": None}
        mod.set_axon_ntff_profile_hook = lambda h: holder.__setitem__("hook", h)
        mod.get_axon_ntff_profile_hook = lambda: holder["hook"]
        sys.modules["antenv.axon_hooks"] = mod
        antenv.axon_hooks = mod
        axon_site = "/root/.axon_site"
        if os.path.isdir(axon_site) and axon_site not in sys.path:
            sys.path.insert(0, axon_site)
        from trn_agent_boot.trn_boot import _ntff_profile_via_ctypes

        hook = _ntff_profile_via_ctypes("/opt/axon/libaxon_pjrt.so")
        if hook is not None:
            mod.set_axon_ntff_profile_hook(hook)
    except Exception as e:  # profiling is best-effort
        print(f"[ntff hook unavailable: {type(e).__name__}: {e}]", flush=True)


def run(x, W_attn, b_attn, W_proj, b_proj, trace=False):
    if trace:
        _ensure_ntff_hook()
        import concourse.bass_utils as _bu

        _bu.upload_artifacts = lambda tmpdir: f"local://{tmpdir}"
    b_attn = np.asarray(b_attn, dtype=np.float32)
    b_proj = np.asarray(b_proj, dtype=np.float32)
    with_bias_qk = bool(np.any(b_attn[: 2 * C] != 0.0))
    with_bias_v = bool(np.any(b_attn[2 * C :] != 0.0))
    nc = _get_program(with_bias_qk, with_bias_v)
    in_maps = _make_in_maps(x, W_attn, b_attn, W_proj)
    res = run_bass_kernel_spmd(
        nc, in_maps, list(range(N_CORES)), trace=trace
    )
    acc = np.zeros((BT, C), dtype=np.float32)
    for r in res.results:
        acc += np.asarray(r["out"], dtype=np.float32)
    acc += b_proj[None, :]
    return acc.reshape(B, T, C).astype(np.float32), res


def kernel(x, W_attn, b_attn, W_proj, b_proj):
    out, _ = run(x, W_attn, b_attn, W_proj, b_proj, trace=False)
    return out
